# revision 17
# baseline (speedup 1.0000x reference)
"""CRF forward (log-partition) kernel for Trainium2, 8 NeuronCores.

Problem: T=16384 steps, NT=1024 tags.
  alpha_0 = strans + emit[0]
  alpha_t[k] = emit[t,k] + logsumexp_j(alpha_{t-1}[j] + trans[j,k])
  out = logsumexp(alpha_{T-1} + etrans) - gold_path_score

Algorithm (validated in fp64/fp8 numpy prototype):
  Work in exp space: with Mc[k]=max_j trans[j,k], Ehat=exp(trans-Mc) in (0,1],
  ghat_t = exp(emit[t]+Mc-mu_t), mu_t = max_k(emit[t]+Mc) + RBAR, the scan is a
  LINEAR recurrence  b_t = ghat_t * (Ehat^T b_{t-1})  whose scalar offsets are
  tracked exactly on the host.  The sequence is cut into 2048 chunks of L=8
  steps; chunk i's chain starts from the seed ghat[8i] standing in for
  b_{8i-1} (positive-matrix products contract directions ~20-30x/step, so by
  the chunk's end the direction is true).  Per-chunk unknown scalars are
  recovered on the host by telescoping ratios: the seed u_i is host-known
  exactly (it IS the fp8 b_init input), v_i is the dumped end-of-chunk state.
  Chain 0 is exact (b_0 injected via a K=1 bf16 matmul patch into PSUM at
  tau=1; its seed column is zero so only the patch contributes).

  Speed: matmuls run fp8 with perf_mode=DoubleRow — each instruction
  contracts K=256 (two 128-blocks packed per PE cell, 2 MACs/cell/cycle):
  per micro-step, 8 output blocks x 4 pair-matmuls of [128x(2x128)] e4m3
  weights against the [128x(2x256)] e5m2 moving b tile (e5m2 because the
  per-chain scale drifts ~10 logs over a chunk, overflowing e4m3's range),
  accumulating fp32 in PSUM, then 8 DVE multiplies by e4m3 ghat writing the
  next e5m2 b tile.  256 chains/core x 8 cores, W=8 micro-steps, no
  inter-core communication.  b tiles are split into two half-tiles (j-blocks
  0-3 / 4-7) so the next micro-step's first matmuls only wait on half the
  DVE writes.  Measured sustained rate ~124 ns per DR matmul.

Chain schedule: chain i>=1 covers steps [8i, 8i+8) at taus 1..8 (no burn-in
micro-step; the seed itself is u_i).  Chain 0 covers steps 1..7 at taus
2..8, patch at tau=1.  Host fixup (fp64): C_0=0,
C_i = C_{i-1} + log sum(v_{i-1}) - log sum(u_i);
logZ = log(v_last . exp(etrans)) + C_last + c_off.
"""

import numpy as np

T, NT = 16384, 1024
NCORES = 8
CH = 256            # chains per core
L = 16384 // (NCORES * CH)   # chunk length = 8
W = L               # micro-steps (no burn-in)
PATCH_TAU = 1       # patch applied at this micro-step
V_TAU = W
RBAR = 1.0          # per-step growth fold-in; centers the per-chain scale
                    # drift (measured [3.9e-3, 64] over a chunk) in e5m2 range

_CACHE = {}


def _build_nc():
    import concourse.bass as bass
    import concourse.mybir as mybir
    import concourse.tile as tile
    from concourse import bacc

    nc = bacc.Bacc("TRN2", target_bir_lowering=False, debug=False,
                   num_devices=NCORES)
    bf16 = mybir.dt.bfloat16
    f32 = mybir.dt.float32
    f8w = mybir.dt.float8e4      # weights + ghat: e4m3 precision
    f8b = mybir.dt.float8e5      # moving b: e5m2 range
    DR = mybir.MatmulPerfMode.DoubleRow

    EH = nc.dram_tensor("ehat", [128, 8, 8, 128], f8w, kind="ExternalInput")
    GH = nc.dram_tensor("ghat", [128, W, 8 * CH], f8w, kind="ExternalInput")
    BI = nc.dram_tensor("binit", [128, 8 * CH], f8b, kind="ExternalInput")
    PA = nc.dram_tensor("patch", [1, 1024], bf16, kind="ExternalInput")
    DV = nc.dram_tensor("dv", [2, 128, 4 * CH], f8b, kind="ExternalOutput")

    with tile.TileContext(nc) as tc:
        with (
            tc.tile_pool(name="const", bufs=1) as const,
            tc.tile_pool(name="bpool", bufs=2) as bpool,
            tc.tile_pool(name="psum", bufs=6, space="PSUM") as psum,
        ):
            # Init loads as few large DMAs (each dma_start costs ~610ns of
            # Sync-queue issue time and a semaphore, which also lengthens the
            # end-of-kernel semaphore-clear chain).  Order = first consumer
            # order; no PE warmup — the first real matmuls warm the HAM while
            # the tail of G streams in.
            patch_sb = const.tile([1, 1024], bf16)
            nc.sync.dma_start(patch_sb[:], PA[:])
            binit_sb = const.tile([128, 8, CH], f8b)
            nc.sync.dma_start(binit_sb[:], BI[:])
            ehall = const.tile([128, 8, 8, 128], f8w)
            nc.sync.dma_start(ehall[:], EH[:])
            g12 = [None, None]
            for i in range(2):
                gt = const.tile([128, 8, CH], f8w, name=f"g{i + 1}")
                nc.sync.dma_start(gt[:], GH[:, i, :])
                g12[i] = gt
            g38 = const.tile([128, W - 2, 8, CH], f8w)
            nc.sync.dma_start(g38[:], GH[:, 2:W, :])
            onehot = const.tile([1, CH], bf16)
            nc.any.memset(onehot[:], 0.0)
            nc.any.memset(onehot[0:1, 0:1], 1.0)

            cur_b = None
            for tau in range(1, W + 1):
                new_b = [bpool.tile([128, 4, CH], f8b, tag=f"b{h}",
                                    name=f"b{tau}_{h}") for h in range(2)]
                for mth in range(8):
                    ps = psum.tile([128, CH], f32, tag="ps")
                    for q in range(4):
                        if tau == 1:
                            rhs = binit_sb[:, 2 * q:2 * q + 2, :]
                        else:
                            rhs = cur_b[q // 2][:, 2 * (q % 2):2 * (q % 2) + 2, :]
                        nc.tensor.matmul(
                            ps[:],
                            lhsT=ehall[:, mth, 2 * q:2 * q + 2, :],
                            rhs=rhs,
                            start=(q == 0),
                            stop=(q == 3 and tau != PATCH_TAU),
                            perf_mode=DR)
                    if tau == PATCH_TAU:
                        nc.tensor.matmul(
                            ps[:],
                            lhsT=patch_sb[:, mth * 128:(mth + 1) * 128],
                            rhs=onehot[:],
                            start=False, stop=True)
                    if tau <= 2:
                        gsl = g12[tau - 1][:, mth, :]
                    else:
                        gsl = g38[:, tau - 3, mth, :]
                    nc.vector.tensor_tensor(
                        out=new_b[mth // 4][:, mth % 4, :], in0=ps[:],
                        in1=gsl,
                        op=mybir.AluOpType.mult)
                if tau == V_TAU:
                    nc.sync.dma_start(DV[0], new_b[0][:])
                    nc.sync.dma_start(DV[1], new_b[1][:])
                cur_b = new_b

    nc.compile()
    return nc


def _get_nc():
    if "nc" not in _CACHE:
        _CACHE["nc"] = _build_nc()
    return _CACHE["nc"]


def _chain_steps():
    """steps[i, tau-1] = global step processed by chain i at micro-step tau
    (-1 = pad)."""
    steps = np.full((NCORES * CH, W), -1, dtype=np.int64)
    taus = np.arange(1, W + 1)
    steps[0] = np.where(taus > PATCH_TAU, taus - PATCH_TAU, -1)
    idx = np.arange(1, NCORES * CH)
    steps[1:] = L * idx[:, None] + taus[None, :] - PATCH_TAU
    return steps


def _preprocess(emit, trans, strans):
    import ml_dtypes
    bf16 = ml_dtypes.bfloat16
    f8w = ml_dtypes.float8_e4m3
    f8b = ml_dtypes.float8_e5m2

    emit64 = emit.astype(np.float64)
    trans64 = trans.astype(np.float64)
    Mc = trans64.max(axis=0)
    Ehat = np.exp(trans64 - Mc[None, :]).astype(np.float32)
    # eh[p, mth, jc, q] = Ehat[jc*128+p, mth*128+q]  (partition-major so the
    # whole array is one contiguous DMA)
    eh = np.ascontiguousarray(
        Ehat.reshape(8, 128, 8, 128).transpose(1, 2, 0, 3)
    ).astype(f8w)

    A = emit64 + Mc[None, :]
    mu = A.max(axis=1) + RBAR                       # [T]
    ghat = np.exp(A - mu[:, None]).astype(np.float32)   # [T, NT]

    a0 = strans.astype(np.float64) + emit64[0]
    c0 = a0.max()
    b0 = np.exp(a0 - c0).astype(np.float32)
    c_off = c0 + mu[1:].sum()

    steps = _chain_steps()
    in_maps = []
    us_all = np.zeros(NCORES * CH)
    for c in range(NCORES):
        S = steps[c * CH:(c + 1) * CH]              # [CH, W]
        G = ghat[np.clip(S, 0, T - 1)]              # [CH, W, NT]
        G = np.where((S >= 1)[:, :, None], G, 0.0)
        if c == 0:
            # chain 0 pad: ghat=1 at PATCH_TAU so the PSUM-injected patch
            # passes through the multiply unchanged
            G[0, PATCH_TAU - 1, :] = 1.0
        # GH[tau, p, blk*CH+ch] = ghat[t_ch, blk*128+p]
        Gt = (G.transpose(1, 2, 0)                  # [W, NT, CH]
                .reshape(W, 8, 128, CH)
                .transpose(0, 2, 1, 3)
                .reshape(W, 128, 8 * CH))
        gh = np.ascontiguousarray(Gt.transpose(1, 0, 2)).astype(f8w)
        # seeds: e5m2-quantized tau-1 ghat columns; chain 0's seed is zero
        # (the patch matmul ADDS b_0 into PSUM, so any seed would leak).
        bi32 = Gt[0].astype(np.float32)
        if c == 0:
            bi32.reshape(128, 8, CH)[:, :, 0] = 0.0
        bi = bi32.astype(f8b)
        # u_i = seed sums, computed exactly from the fp8 input itself
        us_all[c * CH:(c + 1) * CH] = (
            bi.astype(np.float64).reshape(128, 8, CH).sum(axis=(0, 1)))

        pa = np.zeros((1, 1024), np.float32)
        if c == 0:
            pa[0] = b0                              # k-ordered patch row
        in_maps.append({"ehat": np.asarray(eh),
                        "ghat": np.asarray(gh),
                        "binit": np.asarray(bi),
                        "patch": pa.astype(bf16)})
    return in_maps, c_off, us_all


def _postprocess(results, etrans, c_off, us_all):
    """Telescoping seam corrections in fp64."""
    n = NCORES * CH
    Vs = np.zeros(n)
    v_last = None
    for c in range(NCORES):
        dv = (results[c]["dv"].astype(np.float64)
              .reshape(2, 128, 4, CH).transpose(1, 0, 2, 3)
              .reshape(128, 8, CH))
        Vs[c * CH:(c + 1) * CH] = dv.sum(axis=(0, 1))
        if c == NCORES - 1:
            # v[k = blk*128+p] of last chain = dv[p, blk, CH-1]
            v_last = dv[:, :, CH - 1].T.reshape(NT)
    C = (np.log(Vs[:-1]) - np.log(us_all[1:])).sum()
    logZ = np.log((v_last * np.exp(etrans.astype(np.float64))).sum()) + C + c_off
    return logZ


def _score(emit, y, trans, strans, etrans):
    y = y.astype(np.int64)
    return (float(strans[y[0]])
            + trans[y[:-1], y[1:]].astype(np.float64).sum()
            + float(etrans[y[-1]])
            + emit[np.arange(T), y].astype(np.float64).sum())


def _ensure_axon_hooks():
    """Some images lack antenv.axon_hooks; bass_utils imports it whenever
    BASS_TRACE is set under axon.  Provide a no-op shim so kernel() never
    crashes on that path (tracing degrades gracefully)."""
    try:
        import antenv.axon_hooks  # noqa: F401
    except ImportError:
        import sys
        import types
        m = types.ModuleType("antenv.axon_hooks")
        state = {"v": None}
        m.get_axon_ntff_profile_hook = lambda: state["v"]
        m.set_axon_ntff_profile_hook = lambda v: state.update(v=v)
        sys.modules["antenv.axon_hooks"] = m


def kernel(emit, y, trans, strans, etrans):
    _ensure_axon_hooks()
    from concourse.bass_utils import run_bass_kernel_spmd

    emit = np.asarray(emit)
    trans = np.asarray(trans)
    strans = np.asarray(strans)
    etrans = np.asarray(etrans)
    y = np.asarray(y)

    nc = _get_nc()
    in_maps, c_off, us_all = _preprocess(emit, trans, strans)
    res = run_bass_kernel_spmd(nc, in_maps, list(range(NCORES)))
    _CACHE["last_res"] = res
    logZ = _postprocess(res.results, etrans, c_off, us_all)
    out = logZ - _score(emit, y, trans, strans, etrans)
    return np.asarray(out, dtype=np.float32)


# revision 18
# speedup vs baseline: 1.0281x; 1.0281x over previous
"""CRF forward (log-partition) kernel for Trainium2, 8 NeuronCores.

Problem: T=16384 steps, NT=1024 tags.
  alpha_0 = strans + emit[0]
  alpha_t[k] = emit[t,k] + logsumexp_j(alpha_{t-1}[j] + trans[j,k])
  out = logsumexp(alpha_{T-1} + etrans) - gold_path_score

Algorithm (validated in fp64/fp8 numpy prototype):
  Work in exp space: with Mc[k]=max_j trans[j,k], Ehat=exp(trans-Mc) in (0,1],
  ghat_t = exp(emit[t]+Mc-mu_t), mu_t = max_k(emit[t]+Mc) + RBAR, the scan is a
  LINEAR recurrence  b_t = ghat_t * (Ehat^T b_{t-1})  whose scalar offsets are
  tracked exactly on the host.  The sequence is cut into 2048 chunks of L=8
  steps; chunk i's chain starts from the seed ghat[8i] standing in for
  b_{8i-1} (positive-matrix products contract directions ~20-30x/step, so by
  the chunk's end the direction is true).  Per-chunk unknown scalars are
  recovered on the host by telescoping ratios: the seed u_i is host-known
  exactly (it IS the fp8 b_init input), v_i is the dumped end-of-chunk state.
  Chain 0 is exact (b_0 injected via a K=1 bf16 matmul patch into PSUM at
  tau=1; its seed column is zero so only the patch contributes).

  Speed: matmuls run fp8 with perf_mode=DoubleRow — each instruction
  contracts K=256 (two 128-blocks packed per PE cell, 2 MACs/cell/cycle):
  per micro-step, 8 output blocks x 4 pair-matmuls of [128x(2x128)] e4m3
  weights against the [128x(2x256)] e5m2 moving b tile (e5m2 because the
  per-chain scale drifts ~10 logs over a chunk, overflowing e4m3's range),
  accumulating fp32 in PSUM, then 8 DVE multiplies by e4m3 ghat writing the
  next e5m2 b tile.  256 chains/core x 8 cores, W=8 micro-steps, no
  inter-core communication.  b tiles are split into two half-tiles (j-blocks
  0-3 / 4-7) so the next micro-step's first matmuls only wait on half the
  DVE writes.  Measured sustained rate ~124 ns per DR matmul.

Chain schedule: chain i>=1 covers steps [8i, 8i+8) at taus 1..8 (no burn-in
micro-step; the seed itself is u_i).  Chain 0 covers steps 1..7 at taus
2..8, patch at tau=1.  Host fixup (fp64): C_0=0,
C_i = C_{i-1} + log sum(v_{i-1}) - log sum(u_i);
logZ = log(v_last . exp(etrans)) + C_last + c_off.
"""

import numpy as np

T, NT = 16384, 1024
NCORES = 8
CH = 256            # chains per core
L = 16384 // (NCORES * CH)   # chunk length = 8
W = L               # micro-steps (no burn-in)
PATCH_TAU = 1       # patch applied at this micro-step
V_TAU = W
RBAR = 1.0          # per-step growth fold-in; centers the per-chain scale
                    # drift (measured [3.9e-3, 64] over a chunk) in e5m2 range

_CACHE = {}


def _build_nc():
    import concourse.bass as bass
    import concourse.mybir as mybir
    import concourse.tile as tile
    from concourse import bacc

    nc = bacc.Bacc("TRN2", target_bir_lowering=False, debug=False,
                   num_devices=NCORES)
    bf16 = mybir.dt.bfloat16
    f32 = mybir.dt.float32
    f8w = mybir.dt.float8e4      # weights + ghat: e4m3 precision
    f8b = mybir.dt.float8e5      # moving b: e5m2 range
    DR = mybir.MatmulPerfMode.DoubleRow

    EH = nc.dram_tensor("ehat", [128, 8, 8, 128], f8w, kind="ExternalInput")
    GH = nc.dram_tensor("ghat", [128, W, 8 * CH], f8w, kind="ExternalInput")
    BI = nc.dram_tensor("binit", [128, 8 * CH], f8b, kind="ExternalInput")
    PA = nc.dram_tensor("patch", [1, 1024], bf16, kind="ExternalInput")
    DV = nc.dram_tensor("dv", [2, 128, 4 * CH], f8b, kind="ExternalOutput")

    with tile.TileContext(nc) as tc:
        with (
            tc.tile_pool(name="const", bufs=1) as const,
            tc.tile_pool(name="bpool", bufs=2) as bpool,
            tc.tile_pool(name="gpool", bufs=4) as gpool,
            tc.tile_pool(name="psum", bufs=6, space="PSUM") as psum,
        ):
            # Init loads: fine-grained tiles so consumers unblock as each
            # lands (DMA engines round-robin the bandwidth, so one big load
            # completes only when everything does).  Triggers cost ~610ns of
            # issue time each, so they are spread across the three
            # DMA-capable queues (Sync, Scalar/ACT, GpSimd).  No PE warmup:
            # the first real matmuls warm the HAM.
            patch_sb = const.tile([1, 1024], bf16)
            nc.sync.dma_start(patch_sb[:], PA[:])
            binit_sb = const.tile([128, 8, CH], f8b)
            nc.sync.dma_start(binit_sb[:], BI[:])
            ehat_sb = [const.tile([128, 8, 128], f8w, tag=f"eh{m}",
                                  name=f"ehat{m}") for m in range(8)]
            nc.sync.dma_start(ehat_sb[0][:], EH[:, 0])
            g_tiles = {1: gpool.tile([128, 8, CH], f8w, tag="g", name="g1")}
            nc.sync.dma_start(g_tiles[1][:], GH[:, 0, :])
            for mth in range(1, 5):
                nc.scalar.dma_start(ehat_sb[mth][:], EH[:, mth])
            for mth in range(5, 8):
                nc.gpsimd.dma_start(ehat_sb[mth][:], EH[:, mth])
            g_tiles[2] = gpool.tile([128, 8, CH], f8w, tag="g", name="g2")
            nc.gpsimd.dma_start(g_tiles[2][:], GH[:, 1, :])
            onehot = const.tile([1, CH], bf16)
            nc.any.memset(onehot[:], 0.0)
            nc.any.memset(onehot[0:1, 0:1], 1.0)

            cur_b = [binit_sb[:, 0:4, :], binit_sb[:, 4:8, :]]
            for tau in range(1, W + 1):
                g_tile = g_tiles.pop(tau)
                if tau + 2 <= W:
                    nt = gpool.tile([128, 8, CH], f8w, tag="g",
                                    name=f"g{tau + 2}")
                    nc.scalar.dma_start(nt[:], GH[:, tau + 1, :])
                    g_tiles[tau + 2] = nt
                new_b = [bpool.tile([128, 4, CH], f8b, tag=f"b{h}",
                                    name=f"b{tau}_{h}") for h in range(2)]
                for mth in range(8):
                    ps = psum.tile([128, CH], f32, tag="ps")
                    for q in range(4):
                        nc.tensor.matmul(
                            ps[:],
                            lhsT=ehat_sb[mth][:, 2 * q:2 * q + 2, :],
                            rhs=cur_b[q // 2][:, 2 * (q % 2):2 * (q % 2) + 2, :],
                            start=(q == 0),
                            stop=(q == 3 and tau != PATCH_TAU),
                            perf_mode=DR)
                    if tau == PATCH_TAU:
                        nc.tensor.matmul(
                            ps[:],
                            lhsT=patch_sb[:, mth * 128:(mth + 1) * 128],
                            rhs=onehot[:],
                            start=False, stop=True)
                    nc.vector.tensor_tensor(
                        out=new_b[mth // 4][:, mth % 4, :], in0=ps[:],
                        in1=g_tile[:, mth, :],
                        op=mybir.AluOpType.mult)
                if tau == V_TAU:
                    nc.sync.dma_start(DV[0], new_b[0][:])
                    nc.scalar.dma_start(DV[1], new_b[1][:])
                cur_b = [new_b[0][:, :, :], new_b[1][:, :, :]]

    nc.compile()
    return nc


def _get_nc():
    if "nc" not in _CACHE:
        _CACHE["nc"] = _build_nc()
    return _CACHE["nc"]


def _chain_steps():
    """steps[i, tau-1] = global step processed by chain i at micro-step tau
    (-1 = pad)."""
    steps = np.full((NCORES * CH, W), -1, dtype=np.int64)
    taus = np.arange(1, W + 1)
    steps[0] = np.where(taus > PATCH_TAU, taus - PATCH_TAU, -1)
    idx = np.arange(1, NCORES * CH)
    steps[1:] = L * idx[:, None] + taus[None, :] - PATCH_TAU
    return steps


def _preprocess(emit, trans, strans):
    import ml_dtypes
    bf16 = ml_dtypes.bfloat16
    f8w = ml_dtypes.float8_e4m3
    f8b = ml_dtypes.float8_e5m2

    emit64 = emit.astype(np.float64)
    trans64 = trans.astype(np.float64)
    Mc = trans64.max(axis=0)
    Ehat = np.exp(trans64 - Mc[None, :]).astype(np.float32)
    # eh[p, mth, jc, q] = Ehat[jc*128+p, mth*128+q]  (partition-major so the
    # whole array is one contiguous DMA)
    eh = np.ascontiguousarray(
        Ehat.reshape(8, 128, 8, 128).transpose(1, 2, 0, 3)
    ).astype(f8w)

    A = emit64 + Mc[None, :]
    mu = A.max(axis=1) + RBAR                       # [T]
    ghat = np.exp(A - mu[:, None]).astype(np.float32)   # [T, NT]

    a0 = strans.astype(np.float64) + emit64[0]
    c0 = a0.max()
    b0 = np.exp(a0 - c0).astype(np.float32)
    c_off = c0 + mu[1:].sum()

    steps = _chain_steps()
    in_maps = []
    us_all = np.zeros(NCORES * CH)
    for c in range(NCORES):
        S = steps[c * CH:(c + 1) * CH]              # [CH, W]
        G = ghat[np.clip(S, 0, T - 1)]              # [CH, W, NT]
        G = np.where((S >= 1)[:, :, None], G, 0.0)
        if c == 0:
            # chain 0 pad: ghat=1 at PATCH_TAU so the PSUM-injected patch
            # passes through the multiply unchanged
            G[0, PATCH_TAU - 1, :] = 1.0
        # GH[tau, p, blk*CH+ch] = ghat[t_ch, blk*128+p]
        Gt = (G.transpose(1, 2, 0)                  # [W, NT, CH]
                .reshape(W, 8, 128, CH)
                .transpose(0, 2, 1, 3)
                .reshape(W, 128, 8 * CH))
        gh = np.ascontiguousarray(Gt.transpose(1, 0, 2)).astype(f8w)
        # seeds: e5m2-quantized tau-1 ghat columns; chain 0's seed is zero
        # (the patch matmul ADDS b_0 into PSUM, so any seed would leak).
        bi32 = Gt[0].astype(np.float32)
        if c == 0:
            bi32.reshape(128, 8, CH)[:, :, 0] = 0.0
        bi = bi32.astype(f8b)
        # u_i = seed sums, computed exactly from the fp8 input itself
        us_all[c * CH:(c + 1) * CH] = (
            bi.astype(np.float64).reshape(128, 8, CH).sum(axis=(0, 1)))

        pa = np.zeros((1, 1024), np.float32)
        if c == 0:
            pa[0] = b0                              # k-ordered patch row
        in_maps.append({"ehat": np.asarray(eh),
                        "ghat": np.asarray(gh),
                        "binit": np.asarray(bi),
                        "patch": pa.astype(bf16)})
    return in_maps, c_off, us_all


def _postprocess(results, etrans, c_off, us_all):
    """Telescoping seam corrections in fp64."""
    n = NCORES * CH
    Vs = np.zeros(n)
    v_last = None
    for c in range(NCORES):
        dv = (results[c]["dv"].astype(np.float64)
              .reshape(2, 128, 4, CH).transpose(1, 0, 2, 3)
              .reshape(128, 8, CH))
        Vs[c * CH:(c + 1) * CH] = dv.sum(axis=(0, 1))
        if c == NCORES - 1:
            # v[k = blk*128+p] of last chain = dv[p, blk, CH-1]
            v_last = dv[:, :, CH - 1].T.reshape(NT)
    C = (np.log(Vs[:-1]) - np.log(us_all[1:])).sum()
    logZ = np.log((v_last * np.exp(etrans.astype(np.float64))).sum()) + C + c_off
    return logZ


def _score(emit, y, trans, strans, etrans):
    y = y.astype(np.int64)
    return (float(strans[y[0]])
            + trans[y[:-1], y[1:]].astype(np.float64).sum()
            + float(etrans[y[-1]])
            + emit[np.arange(T), y].astype(np.float64).sum())


def _ensure_axon_hooks():
    """Some images lack antenv.axon_hooks; bass_utils imports it whenever
    BASS_TRACE is set under axon.  Provide a no-op shim so kernel() never
    crashes on that path (tracing degrades gracefully)."""
    try:
        import antenv.axon_hooks  # noqa: F401
    except ImportError:
        import sys
        import types
        m = types.ModuleType("antenv.axon_hooks")
        state = {"v": None}
        m.get_axon_ntff_profile_hook = lambda: state["v"]
        m.set_axon_ntff_profile_hook = lambda v: state.update(v=v)
        sys.modules["antenv.axon_hooks"] = m


def kernel(emit, y, trans, strans, etrans):
    _ensure_axon_hooks()
    from concourse.bass_utils import run_bass_kernel_spmd

    emit = np.asarray(emit)
    trans = np.asarray(trans)
    strans = np.asarray(strans)
    etrans = np.asarray(etrans)
    y = np.asarray(y)

    nc = _get_nc()
    in_maps, c_off, us_all = _preprocess(emit, trans, strans)
    res = run_bass_kernel_spmd(nc, in_maps, list(range(NCORES)))
    _CACHE["last_res"] = res
    logZ = _postprocess(res.results, etrans, c_off, us_all)
    out = logZ - _score(emit, y, trans, strans, etrans)
    return np.asarray(out, dtype=np.float32)
